# revision 40
# baseline (speedup 1.0000x reference)
"""Trainium2 Bass kernel for nn_MultiHeadAttention_59614146068609.

Sharding: 8 cores = 2 batches x 4 head-groups (4 heads each).
Each core projects q/k/v for its batch with its head-slice of Wq/Wk/Wv
(column-sharded), runs causal+padded attention for its 4 heads, and
applies its row-slice of Wo, producing a partial [D, S] fp16 output.
The host sums the 4 partials per batch and adds bo (with Wo @ bv folded
in on the host: softmax weights sum to 1, so attn(v + bv) = attn(v) + bv).

Layout: q/k land transposed and PAIR-PACKED ([128, pair, s] with head
2p in partitions 0:64 and head 2p+1 in 64:128), so projections evict
full-width tiles and attention runs 64-contraction matmuls at partition
offsets 0/64 (PE quadrant tile_position). Scores for a head-pair land
in one 2-bank PSUM tile [128, 2, 512] so a single ACT exp instruction
covers both heads (the scalar engine is the scarce resource). V is
natural layout with an appended ones-column providing softmax sums.
The causal mask is applied AFTER exp as a 0/1 multiply on the fp16
probability tile, keeping the vector engine out of the scores->exp
critical chain.

Schedule: one software pipeline. Attention for chunk qc interleaves,
per key-block step, "filler" tensor work units (q/k/v projections for
qc+1 and the Wo output projection of qc-1) popped from a queue, so the
tensor engine always streams (TRN2 PE p-state needs continuous
execution for 2.4 GHz). PV trails scores by one key block. All inputs
are host-prepacked so every DMA is per-partition contiguous.

Specialized at build time on kb_cap = number of 128-wide key blocks
that contain any unpadded key; fully padded key blocks are skipped.
"""

from collections import deque

import numpy as np

S = 2048
B = 2
D = 1024
H = 16
DK = 64
N_CORES = 8
GROUPS = N_CORES // B          # head groups per batch = 4
HPG = H // GROUPS              # heads per group = 4
OC = HPG * DK                  # per-core projected dim = 256
OT = OC // 128                 # o-tiles / head-pairs per core = 2
IT = D // 128                  # contraction tiles = 8
SC = S // 512                  # sequence chunks of 512 = 4
KB = S // 128                  # k blocks of 128 = 16
NEG = -1e30

_cache = {}


def _build_nc(kb_cap):
    import concourse.bacc as bacc
    import concourse.bass as bass
    import concourse.mybir as mybir
    import concourse.tile as tile
    from concourse import library_config

    F32 = mybir.dt.float32
    FP16 = mybir.dt.float16
    Exp = mybir.ActivationFunctionType.Exp
    PSUM = bass.MemorySpace.PSUM

    ksc = -(-kb_cap * 128 // 512)        # 512-chunks of k to project
    nkb = [min(4 * (qc + 1), kb_cap) for qc in range(SC)]

    def vblocks(qc):
        # v key-blocks first needed by attention chunk qc
        if qc >= SC:
            return []
        return list(range(4 * qc, min(4 * (qc + 1), kb_cap)))

    nc = bacc.Bacc("TRN2", target_bir_lowering=False, debug=False)

    # all inputs host-prepacked: partition-major, chunk-contiguous
    xq = nc.dram_tensor("xq", [128, SC, IT, 512], FP16, kind="ExternalInput")
    xk = nc.dram_tensor("xk", [128, SC, IT, 512], FP16, kind="ExternalInput")
    xv = nc.dram_tensor("xv", [128, SC, IT, 512], FP16, kind="ExternalInput")
    wq = nc.dram_tensor("wq", [128, IT, OC], FP16, kind="ExternalInput")
    wk = nc.dram_tensor("wk", [128, IT, OC], FP16, kind="ExternalInput")
    wv = nc.dram_tensor("wv", [128, IT, OC], FP16, kind="ExternalInput")
    wo = nc.dram_tensor("wo", [128, OT, D], FP16, kind="ExternalInput")
    # consts: cols 0:2 = scaled bq (per o-tile), 2:4 = bk, 4:20 = pad bias
    consts = nc.dram_tensor("consts", [128, 20], F32, kind="ExternalInput")
    mask01 = nc.dram_tensor("mask01", [128, 2, 128], FP16, kind="ExternalInput")
    out_t = nc.dram_tensor("out_t", [D, S], FP16, kind="ExternalOutput")

    with tile.TileContext(nc) as tc, nc.allow_low_precision(
        reason="fp16 compute throughout; validated vs fp32 reference"
    ):
        with (
            tc.tile_pool(name="persist", bufs=1) as pp,
            tc.tile_pool(name="xs", bufs=2) as xs,
            tc.tile_pool(name="ptp", bufs=26) as ptp,
            tc.tile_pool(name="nrm", bufs=2) as nrm,
            tc.tile_pool(name="stg", bufs=4) as stg,
            tc.tile_pool(name="ps", bufs=2, space=PSUM) as ps,
        ):
            # ---- persistent SBUF tensors ----
            t_wq = pp.tile([128, IT, OC], FP16)
            t_wk = pp.tile([128, IT, OC], FP16)
            t_wv = pp.tile([128, IT, OC], FP16)
            t_wo = pp.tile([128, OT, D], FP16)
            t_c = pp.tile([128, 20], F32)
            t_mask = pp.tile([128, 2, 128], FP16)
            t_qT = pp.tile([128, OT, S], FP16)
            t_kT = pp.tile([128, OT, ksc * 512], FP16)
            t_V = pp.tile([128, kb_cap, HPG, DK + 1], FP16)
            t_OT = pp.tile([128, OT, S], FP16)

            # DMA load split across all three issuing queues (each sprays
            # over 16 DMA engines): sync=xq+consts, scalar=weights+xk,
            # gpsimd=xv+output. Ordered by first use.
            nc.scalar.dma_start(out=t_wq[:, 0:4, :], in_=wq[:, 0:4, :])
            nc.scalar.dma_start(out=t_wq[:, 4:8, :], in_=wq[:, 4:8, :])
            nc.scalar.dma_start(out=t_wk[:, 0:4, :], in_=wk[:, 0:4, :])
            nc.scalar.dma_start(out=t_wk[:, 4:8, :], in_=wk[:, 4:8, :])
            # small constants ride the gpsimd queue so they never delay the
            # first xq chunk on sync (first matmul gates on wq + xq0); they
            # must be issued BEFORE the gpsimd library load or the first
            # q-eviction stalls ~4us waiting for t_c behind it
            nc.gpsimd.dma_start(out=t_c, in_=consts[:])
            nc.gpsimd.dma_start(out=t_mask, in_=mask01[:])
            nc.gpsimd.load_library(library_config.attn)
            # softmax-denominator ones column of V
            nc.vector.memset(t_V[:, :, :, DK : DK + 1], 1.0)

            # ---- chunk-granular x DMAs (sync queue) ----
            xq_t = {}
            xk_t = {}
            xv_t = {}

            # each chunk is loaded as 4 sub-DMAs: a single dma_start only
            # engages ~4 of a queue's 16 DMA engines (~90 GB/s), but queued
            # transfers overlap, so splitting restores full queue bandwidth
            def dma_xchunk(sc):
                if sc < SC:
                    t = xs.tile([128, IT, 512], FP16, tag="xq", name=f"xq_{sc}")
                    for h in range(0, IT, 2):
                        nc.sync.dma_start(
                            out=t[:, h : h + 2, :], in_=xq[:, sc, h : h + 2, :]
                        )
                    xq_t[sc] = t
                if sc < ksc:
                    t = xs.tile([128, IT, 512], FP16, tag="xk", name=f"xk_{sc}")
                    for h in range(0, IT, 2):
                        nc.scalar.dma_start(
                            out=t[:, h : h + 2, :], in_=xk[:, sc, h : h + 2, :]
                        )
                    xk_t[sc] = t
                if sc == 0:
                    # remaining weights queue behind wq/wk/xk0 on scalar
                    nc.scalar.dma_start(out=t_wv, in_=wv[:])
                    nc.scalar.dma_start(out=t_wo, in_=wo[:])

            def dma_xv(blocks):
                if not blocks:
                    return
                g = blocks[0] // 4
                t = xs.tile([128, IT, 512], FP16, tag="xv", name=f"xv_{g}")
                for h in range(0, IT, 2):
                    nc.gpsimd.dma_start(
                        out=t[:, h : h + 2, :], in_=xv[:, g, h : h + 2, :]
                    )
                xv_t[g] = t

            # ---- work units (each emits one PE matmul group + eviction) ----
            # filler units get their OWN psum tag "f" (1 bank x 2 bufs) so
            # their matmuls never rotate through the score tiles' banks —
            # otherwise every filler serializes behind the exp chain
            def unit_qk(sc, w_sb, cofs, dst, ot):
                def emit():
                    xt = xq_t[sc] if cofs == 0 else xk_t[sc]
                    acc = ps.tile(
                        [128, 512], F32, tag="f", bufs=2, name=f"a{cofs}_{sc}_{ot}"
                    )
                    for i in range(IT):
                        nc.tensor.matmul(
                            acc,
                            w_sb[:, i, ot * 128 : (ot + 1) * 128],
                            xt[:, i, :],
                            start=(i == 0),
                            stop=(i == IT - 1),
                        )
                    nc.vector.tensor_scalar_add(
                        out=dst[:, ot, sc * 512 : (sc + 1) * 512],
                        in0=acc,
                        scalar1=t_c[:, cofs + ot : cofs + ot + 1],
                    )
                return emit

            def unit_v(n, blk):
                def emit():
                    xt = xv_t[blk // 4]
                    vacc = ps.tile([128, 512], F32, tag="f", bufs=2, name=f"v_{blk}")
                    for i in range(IT):
                        nc.tensor.matmul(
                            vacc[:, 0:OC],
                            xt[:, i, n * 128 : (n + 1) * 128],
                            t_wv[:, i, :],
                            start=(i == 0),
                            stop=(i == IT - 1),
                        )
                    nc.vector.tensor_copy(
                        out=t_V[:, blk, :, 0:DK],
                        in_=vacc[:, 0:OC].rearrange("p (h d) -> p h d", h=HPG),
                    )
                return emit

            def unit_phase_c(qc, dt_, ptag="f"):
                def emit():
                    q0 = qc * 512
                    ops = ps.tile(
                        [128, 512], F32, tag=ptag, bufs=2, name=f"c_{qc}_{dt_}"
                    )
                    for j in range(OT):
                        nc.tensor.matmul(
                            ops,
                            t_wo[:, j, dt_ * 128 : (dt_ + 1) * 128],
                            t_OT[:, j, q0 : q0 + 512],
                            start=(j == 0),
                            stop=(j == OT - 1),
                        )
                    st_o = stg.tile([128, 512], FP16, tag="s", name=f"so_{qc}_{dt_}")
                    # at the tail (final chunk) ACT has no exps left: split
                    # stagings across DVE and ACT to halve the exposed chain
                    if qc == SC - 1 and dt_ % 2 == 1:
                        nc.scalar.copy(st_o, ops)
                    else:
                        nc.vector.tensor_copy(st_o, ops)
                    # spread output writes over all three DMA queues so the
                    # final chunk's writes drain in parallel
                    eng = (nc.gpsimd, nc.sync, nc.scalar)[dt_ % 3]
                    eng.dma_start(
                        out=out_t[dt_ * 128 : (dt_ + 1) * 128, q0 : q0 + 512],
                        in_=st_o,
                    )
                return emit

            def proj_units(sc):
                u = []
                if sc < SC:
                    for ot in range(OT):
                        u.append(unit_qk(sc, t_wq, 0, t_qT, ot))
                if sc < ksc:
                    for ot in range(OT):
                        u.append(unit_qk(sc, t_wk, 2, t_kT, ot))
                for n, blk in enumerate(vblocks(sc)):
                    u.append(unit_v(n, blk))
                return u

            # ---- fused pipelined main loop ----
            dma_xchunk(0)
            dma_xv(vblocks(0))
            dma_xchunk(1)
            dma_xv(vblocks(1))

            fillers = deque()
            pc_stash = {}
            # chunk 0's q/k projections must precede its first scores matmul;
            # its v projections interleave into the loop (PV trails 1 block)
            fillers.extend(proj_units(0))
            n_fill0 = 2 * OT if ksc > 0 else OT

            for qc in range(SC):
                q0 = qc * 512
                # prefetch chunk qc+2 inputs; queue chunk qc+1 projections
                dma_xchunk(qc + 2)
                dma_xv(vblocks(qc + 2))
                if qc + 1 < SC:
                    if qc == 0:
                        # chunk-0's v units are still queued and must pop
                        # first (PV of chunk 0 trails them by one step)
                        fillers.extend(proj_units(1))
                    else:
                        # projections go to the FRONT: their inputs (x
                        # chunks) are long since resident, while the phase-C
                        # units queued behind depend on an earlier chunk's
                        # norm and would head-of-line block the PE
                        fillers.extendleft(reversed(proj_units(qc + 1)))
                # phase-C units scheduled for this chunk's window (delayed
                # two chunks so filler work lands in the later, exp-paced
                # chunks where the PE would otherwise run dry)
                fillers.extend(pc_stash.pop(qc, []))

                last = nkb[qc] - 1
                steps = nkb[qc]

                def alloc_ops(p):
                    return {
                        hh: ps.tile(
                            [128, 512], F32, tag="o", bufs=2, name=f"o_{qc}_{p}_{hh}"
                        )
                        for hh in range(2)
                    }

                def emit_pv(p, o_ps, kb, off, pt):
                    for hh in range(2):
                        nc.tensor.matmul(
                            o_ps[hh][0 : DK + 1, off:512],
                            t_V[:, kb, 2 * p + hh, :],
                            pt[:, hh, off:512],
                            start=(kb == 0),
                            stop=(kb == last),
                        )

                def norm(p, o_ps):
                    # stage-ordered across both heads so the Pool broadcasts
                    # overlap the DVE recip of the other head instead of
                    # serializing copy->recip->bcast->mul per head
                    t_l, t_r, t_rb = {}, {}, {}
                    for hh in range(2):
                        t_l[hh] = nrm.tile(
                            [1, 512], F32, tag="l", name=f"l_{qc}_{p}_{hh}"
                        )
                        nc.vector.tensor_copy(
                            t_l[hh][0:1, :], o_ps[hh][DK : DK + 1, :]
                        )
                    for hh in range(2):
                        t_r[hh] = nrm.tile(
                            [1, 512], F32, tag="r", name=f"r_{qc}_{p}_{hh}"
                        )
                        nc.vector.reciprocal_approx_fast(
                            t_r[hh][0:1, :], t_l[hh][0:1, :]
                        )
                        t_rb[hh] = nrm.tile(
                            [DK, 512], F32, tag="rb", name=f"rb_{qc}_{p}_{hh}"
                        )
                        nc.gpsimd.partition_broadcast(t_rb[hh], t_r[hh][0:1, :])
                    for hh in range(2):
                        nc.vector.tensor_mul(
                            t_OT[hh * 64 : (hh + 1) * 64, p, q0 : q0 + 512],
                            o_ps[hh][0:DK, :],
                            t_rb[hh],
                        )

                if qc == 0:
                    for _ in range(n_fill0):
                        fillers.popleft()()

                # pair 0 accumulates in-loop (PV trails scores by 2 blocks);
                # pair 1's probability tiles are retained in SBUF and swept
                # after pair 0's normalization, halving live PSUM o-banks.
                # The FINAL chunk interleaves pair 1 too (its op1 borrows the
                # filler banks, its phase-C fillers move to tag "w") so no
                # sweep is exposed in the kernel tail.
                inter_p1 = qc == SC - 1
                op0 = alloc_ops(0)
                op1 = None
                if inter_p1:
                    op1 = {
                        hh: ps.tile(
                            [128, 512], F32, tag="f", bufs=2, name=f"o1_{qc}_{hh}"
                        )
                        for hh in range(2)
                    }
                pvq = deque()
                p1blocks = []
                for kb in range(steps):
                    k0 = kb * 128
                    off = max(0, k0 - q0)
                    # fillers FIRST: independent work the PE can always run
                    # even while scores wait for exp to free their banks
                    remaining = steps - kb
                    n_pop = max(1, -(-len(fillers) // remaining)) if fillers else 0
                    for _ in range(min(n_pop, len(fillers))):
                        fillers.popleft()()
                    st2s = {}
                    for p in range(OT):
                        st2 = ps.tile(
                            [128, 2, 512], F32, tag="w", bufs=2,
                            name=f"st_{qc}_{kb}_{p}",
                        )
                        for hh in range(2):
                            nc.tensor.matmul(
                                st2[:, hh, off:512],
                                t_kT[hh * 64 : (hh + 1) * 64, p, k0 : k0 + 128],
                                t_qT[hh * 64 : (hh + 1) * 64, p, q0 + off : q0 + 512],
                                start=True,
                                stop=True,
                            )
                        st2s[p] = st2
                    while len(pvq) > (2 if inter_p1 else 1):
                        it = pvq.popleft()
                        if len(it) == 4:
                            emit_pv(1, op1, *it[:3])
                        else:
                            emit_pv(0, op0, *it)
                    pts = {}
                    for p in range(OT):
                        pt = ptp.tile(
                            [128, 2, 512], FP16, tag="pt", name=f"pt_{qc}_{kb}_{p}"
                        )
                        nc.scalar.activation(
                            out=pt[:, :, off:512],
                            in_=st2s[p][:, :, off:512],
                            func=Exp,
                            bias=t_c[:, 4 + kb : 5 + kb],
                            scale=1.0,
                        )
                        pts[p] = pt
                    if k0 >= q0:
                        # causal mask applied post-exp (0/1 multiply) so the
                        # vector engine stays out of the scores->exp chain
                        for p in range(OT):
                            nc.vector.tensor_mul(
                                pts[p][:, :, off : off + 128],
                                pts[p][:, :, off : off + 128],
                                t_mask,
                            )
                    pvq.append((kb, off, pts[0]))
                    if inter_p1:
                        pvq.append((kb, off, pts[1], 1))
                    else:
                        p1blocks.append((kb, off, pts[1]))
                # flush: emit each pair's remaining PVs, norming a pair the
                # moment its accumulation stops so the norm chain overlaps
                # the other pair's tail work
                p0_left = [it for it in pvq if len(it) == 3]
                p1_left = [it[:3] for it in pvq if len(it) == 4]
                for it in p0_left:
                    emit_pv(0, op0, *it)
                norm(0, op0)
                if inter_p1:
                    for it in p1_left:
                        emit_pv(1, op1, *it)
                else:
                    op1 = alloc_ops(1)
                    for kb, off, pt in p1blocks:
                        emit_pv(1, op1, kb, off, pt)
                norm(1, op1)

                # output projection of this chunk runs interleaved into a
                # LATER chunk's attention; the final chunk's runs at the tail
                if qc + 1 < SC:
                    tgt = min(qc + 2, SC - 1)
                    ptag = "w" if tgt == SC - 1 else "f"
                    pc_stash.setdefault(tgt, []).extend(
                        unit_phase_c(qc, dt_, ptag) for dt_ in range(D // 128)
                    )
                else:
                    for dt_ in range(D // 128):
                        unit_phase_c(qc, dt_, "w")()
    nc.compile()
    return nc


def _get_nc(kb_cap):
    key = ("nc", kb_cap)
    if key not in _cache:
        _cache[key] = _build_nc(kb_cap)
    return _cache[key]


def _prepack_x(x):  # [D, S] -> [128, SC, IT, 512], partition-major chunks
    return np.ascontiguousarray(
        x.reshape(IT, 128, SC, 512).transpose(1, 2, 0, 3)
    )


def kernel(
    query,
    key,
    value,
    Wq,
    bq,
    Wk,
    bk,
    Wv,
    bv,
    Wo,
    bo,
    attn_mask,
    key_padding_mask,
):
    from concourse import bass_utils

    query = np.asarray(query, dtype=np.float32)
    key = np.asarray(key, dtype=np.float32)
    value = np.asarray(value, dtype=np.float32)
    Wq = np.asarray(Wq, dtype=np.float32)
    bq = np.asarray(bq, dtype=np.float32)
    Wk = np.asarray(Wk, dtype=np.float32)
    bk = np.asarray(bk, dtype=np.float32)
    Wv = np.asarray(Wv, dtype=np.float32)
    bv = np.asarray(bv, dtype=np.float32)
    Wo = np.asarray(Wo, dtype=np.float32)
    bo = np.asarray(bo, dtype=np.float32)
    attn_mask = np.asarray(attn_mask)
    key_padding_mask = np.asarray(key_padding_mask)

    # this kernel hardcodes the causal structure of attn_mask
    expected = np.triu(np.ones((S, S), dtype=bool), k=1)
    assert np.array_equal(attn_mask, expected), "kernel assumes causal attn_mask"

    # number of 128-blocks that contain any valid (unpadded) key
    valid = ~key_padding_mask  # [B, S]
    kb_cap = 0
    for b in range(B):
        nz = np.nonzero(valid[b])[0]
        cap = (int(nz.max()) // 128 + 1) if nz.size else 1
        kb_cap = max(kb_cap, cap)

    scale = np.float32(1.0 / np.sqrt(DK))
    m01 = (np.arange(128)[None, :] >= np.arange(128)[:, None]).astype(np.float16)
    mask01 = np.ascontiguousarray(np.stack([m01, m01], axis=1))  # [128, 2, 128]

    # per-batch prepacked activations (shared by the batch's 4 cores)
    xq_b = [_prepack_x(query[:, b, :].T.astype(np.float16)) for b in range(B)]
    xk_b = [_prepack_x(key[:, b, :].T.astype(np.float16)) for b in range(B)]
    xv_b = [_prepack_x(value[:, b, :].T.astype(np.float16)) for b in range(B)]
    pad_b = [
        np.where(key_padding_mask[b], NEG, 0.0).astype(np.float32).reshape(KB, 128).T
        for b in range(B)
    ]

    in_maps = []
    for c in range(N_CORES):
        b = c // GROUPS
        g = c % GROUPS
        o0 = g * OC
        osl = slice(o0, o0 + OC)
        consts = np.zeros((128, 20), dtype=np.float32)
        consts[:, 0:2] = (bq[osl] * scale).reshape(OT, 128).T
        consts[:, 2:4] = bk[osl].reshape(OT, 128).T
        consts[:, 4:20] = pad_b[b]
        in_maps.append(
            {
                "xq": xq_b[b],
                "xk": xk_b[b],
                "xv": xv_b[b],
                "wq": np.ascontiguousarray(
                    (Wq[osl, :] * scale).T.astype(np.float16)
                    .reshape(IT, 128, OC).transpose(1, 0, 2)
                ),
                "wk": np.ascontiguousarray(
                    Wk[osl, :].T.astype(np.float16)
                    .reshape(IT, 128, OC).transpose(1, 0, 2)
                ),
                "wv": np.ascontiguousarray(
                    Wv[osl, :].T.astype(np.float16)
                    .reshape(IT, 128, OC).transpose(1, 0, 2)
                ),
                "wo": np.ascontiguousarray(
                    Wo[:, osl].T.astype(np.float16)
                    .reshape(OT, 128, D).transpose(1, 0, 2)
                ),
                "consts": consts,
                "mask01": mask01,
            }
        )

    res = bass_utils.run_bass_kernel_spmd(
        _get_nc(kb_cap), in_maps, core_ids=list(range(N_CORES))
    )
    _cache["last_res"] = res

    bo2 = bo + Wo @ bv  # softmax weights sum to 1: attn(v+bv) = attn(v)+bv
    out = np.zeros((S, B, D), dtype=np.float32)
    for b in range(B):
        acc = np.zeros((D, S), dtype=np.float32)
        for g in range(GROUPS):
            acc += res.results[b * GROUPS + g]["out_t"].astype(np.float32)
        out[:, b, :] = acc.T + bo2[None, :]
    return out
